# revision 1
# baseline (speedup 1.0000x reference)
"""Trainium2 Bass kernel for LocalSpatioTemporalPooling.

Reference computation (per sample n):
  x: (C=256, T=30, H=64, W=44) fp32
  feats[c,t,s] = mean over the (8,44) spatial stripe s of frame t    # 352-elem mean
  scores[t,s] = || feats[:,t,s] ||_2  (clip eps)                     # reduce over C
  top-2 frames per stripe by score; output[s*C + c] = mean of the 2 selected feats

Sharding: pure data parallel -- one sample per NeuronCore (N=8 = n_cores).

Kernel structure per core (x viewed as (C, T*S*352); 352-groups contiguous in HBM):
  - Stream 17 chunks (c-block 0: 6x5 frames; c-block 1 tapers 5,4,4,4,3,3,2,2,1,1,1
    so each chunk's DVE reduce (~2.95us/frame) fits inside the next chunk's DMA
    window (~4.0us/frame) and the post-DMA tail is just the final 1-frame reduce).
    Each DMA is contiguous per partition.  The 352-group sums are split between
    the DVE (tensor_reduce, ~3.0us/frame) and the otherwise-idle ACT engine
    (accumulate-copy per group, ~3.8us/frame) so per-engine busy time (~100us
    each) stays below the DMA span whether the per-core HBM rate is ~360 GB/s
    (cost-model/"HBM-per-NC" limit -> ~240us stream) or higher.  Results land in
    feats (128, 480) = [c-block 0 | c-block 1], stripe-major (s*30 + t).
  - Scores accumulate DURING the stream: after each chunk's reduce, ACT squares
    that slice and PE accumulates ones^T @ sq into the (1, 240) PSUM sumsq
    (per t-range: c-block 0 matmul starts, c-block 1 stops).  Ordering by sumsq
    == ordering by the reference score (monotonic transform).
  - Top-2 per stripe with no DMA round-trip: per-stripe max via strided
    reduce_max on (1, 8, 30), tie mask via stride-0 broadcast tensor_tensor,
    mask out the max, second reduce_max, then w = (ss >= m2) in bf16.
  - PE broadcast (bf16 ones row) -> wb (128, 480); one scalar_tensor_tensor
    (feats * WSCALE * wb), one strided reduce -> (128, 16), one PE transpose
    -> (16, 128), one copy, one DMA to out (viewed (cb s) c).
"""

import dataclasses

import numpy as np
from contextlib import ExitStack

import concourse.bass as bass
import concourse.tile as tile
import concourse.mybir as mybir
from concourse import bacc
from concourse.bass_utils import run_bass_kernel_spmd
from concourse.masks import make_identity

N, C, T, H, W = 8, 256, 30, 64, 44
S = 8                 # stripes
SH = H // S           # 8 rows per stripe
GROUP = SH * W        # 352 elements per (c, t, s) group
CB = C // 128         # 2 channel blocks
FRAME = H * W         # 2816
WSCALE = 0.5 / GROUP  # top-2 mean of stripe means
BIG = 1.0e30

CHUNKS = [
    [(0, 5), (5, 5), (10, 5), (15, 5), (20, 5), (25, 5)],                # c-block 0
    [(0, 5), (5, 4), (9, 4), (13, 4), (17, 3), (20, 3), (23, 2),
     (25, 2), (27, 1), (28, 1), (29, 1)],                                # c-block 1
]

_F32 = mybir.dt.float32
_BF16 = mybir.dt.bfloat16


def _bcast(ap2d, inner):
    """(1, K) AP -> (1, K, inner) stride-0 broadcast view."""
    [pp, pc], [fs, fc] = ap2d.ap[0], ap2d.ap[1]
    return dataclasses.replace(ap2d, ap=[[pp, pc], [fs, fc], [0, inner]])


def _kernel_body(ctx, tc, nc, x, out, repeat=1):
    const_pool = ctx.enter_context(tc.tile_pool(name="const", bufs=1))
    in_pool = ctx.enter_context(tc.tile_pool(name="inp", bufs=3))
    feat_pool = ctx.enter_context(tc.tile_pool(name="feat", bufs=1))
    small_pool = ctx.enter_context(tc.tile_pool(name="small", bufs=1))
    psum_pool = ctx.enter_context(tc.tile_pool(name="psum", bufs=1, space="PSUM"))

    ones_col = const_pool.tile([128, 1], _F32)
    nc.vector.memset(ones_col[:], 1.0)
    wrow = const_pool.tile([1, 128], _BF16)
    nc.vector.memset(wrow[:], 1.0)
    identity = const_pool.tile([128, 128], _F32)
    make_identity(nc, identity[:])

    # [c-block 0 | c-block 1] side by side; free layout within a block: s*30 + t
    feats = feat_pool.tile([128, CB * T * S], _F32)
    sq = feat_pool.tile([128, CB * T * S], _F32)
    ss_psum = psum_pool.tile([1, T * S], _F32, tag="ss")
    ssv = ss_psum[:].rearrange("p (s t) -> p s t", s=S)

    def fview(cb, t0, tc_, s0=0, sn=S):  # (128, tc_, sn) t-minor slice
        return feats[:, cb * T * S:(cb + 1) * T * S].rearrange(
            "p (s t) -> p t s", s=S)[:, t0:t0 + tc_, s0:s0 + sn]

    # scratch rows for ACT-side accumulate-copy (value discarded, accum kept)
    act_scratch = [feat_pool.tile([128, GROUP], _F32, name=f"actscr{i}")
                   for i in range(2)]
    act_n = [0]

    def act_group_sum(cb, tile_ap, tcn, t0, t_local, s_):
        # sum one (c, t, s) 352-group on the Scalar engine via accum_out
        scr = act_scratch[act_n[0] % 2]
        act_n[0] += 1
        g = t_local * S + s_
        nc.scalar.activation(
            scr[:], tile_ap[:, g * GROUP:(g + 1) * GROUP],
            mybir.ActivationFunctionType.Copy,
            accum_out=feats[:, cb * T * S + s_ * T + (t0 + t_local):
                            cb * T * S + s_ * T + (t0 + t_local) + 1],
        )

    def sview(t_, cb, t0, tc_):  # (128, 8, tc_) s-major slice
        return t_[:, cb * T * S:(cb + 1) * T * S].rearrange(
            "p (s t) -> p s t", s=S)[:, :, t0:t0 + tc_]

    for _rep in range(repeat):
        # ---- streamed reduction + in-stream score accumulation ----
        for cb in range(CB):
            for (t0, tcn) in CHUNKS[cb]:
                tl = in_pool.tile([128, 5 * S * GROUP], _F32, name="tl", tag="tl")
                nc.sync.dma_start(
                    tl[:, :tcn * S * GROUP],
                    x[cb * 128:(cb + 1) * 128, t0 * FRAME:(t0 + tcn) * FRAME],
                )
                in4 = tl[:, :tcn * S * GROUP].rearrange("p (t s w) -> p t s w", t=tcn, s=S)
                # split the group-sums between DVE (tensor_reduce) and the
                # otherwise-idle ACT engine (accumulate-copy per group):
                # DVE ~2.95us/frame vs ACT ~3.84us/frame.
                dve_f, act_f = {5: (3, 2), 4: (2, 2), 3: (2, 1),
                                2: (1, 1), 1: (0, 0)}[tcn]
                if tcn == 1:
                    # half a frame each: DVE takes stripes 0-3, ACT takes 4-7
                    nc.vector.tensor_reduce(
                        fview(cb, t0, 1, 0, 4), in4[:, :, 0:4, :],
                        axis=mybir.AxisListType.X, op=mybir.AluOpType.add,
                    )
                    for s_ in range(4, S):
                        act_group_sum(cb, tl, tcn, t0, 0, s_)
                else:
                    nc.vector.tensor_reduce(
                        fview(cb, t0, dve_f), in4[:, :dve_f, :, :],
                        axis=mybir.AxisListType.X, op=mybir.AluOpType.add,
                    )
                    for tloc in range(dve_f, tcn):
                        for s_ in range(S):
                            act_group_sum(cb, tl, tcn, t0, tloc, s_)
                nc.scalar.activation(
                    sview(sq, cb, t0, tcn), sview(feats, cb, t0, tcn),
                    mybir.ActivationFunctionType.Square,
                )
                if cb == 1:
                    # both c-blocks' squares for this t-range are now available
                    nc.tensor.matmul(
                        ssv[:, :, t0:t0 + tcn], lhsT=ones_col[:],
                        rhs=sview(sq, 0, t0, tcn), start=True, stop=False,
                    )
                    nc.tensor.matmul(
                        ssv[:, :, t0:t0 + tcn], lhsT=ones_col[:],
                        rhs=sview(sq, 1, t0, tcn), start=False, stop=True,
                    )

        # ---- per-stripe top-2 -> weight row (all on partition 0, no DMA) ----
        m1 = small_pool.tile([1, S], _F32)
        nc.vector.tensor_reduce(m1[:], ssv, axis=mybir.AxisListType.X,
                                op=mybir.AluOpType.max)
        eq1 = small_pool.tile([1, T * S], _F32)
        eq1v = eq1[:].rearrange("p (s t) -> p s t", s=S)
        nc.vector.tensor_tensor(eq1v, ssv, _bcast(m1[:], T), op=mybir.AluOpType.is_ge)
        masked = small_pool.tile([1, T * S], _F32)
        maskedv = masked[:].rearrange("p (s t) -> p s t", s=S)
        nc.vector.scalar_tensor_tensor(
            maskedv, eq1v, -BIG, ssv,
            op0=mybir.AluOpType.mult, op1=mybir.AluOpType.add,
        )
        m2 = small_pool.tile([1, S], _F32)
        nc.vector.tensor_reduce(m2[:], maskedv, axis=mybir.AxisListType.X,
                                op=mybir.AluOpType.max)
        w = small_pool.tile([1, T * S], _BF16)
        wv = w[:].rearrange("p (s t) -> p s t", s=S)
        nc.vector.tensor_tensor(wv, ssv, _bcast(m2[:], T), op=mybir.AluOpType.is_ge)

        # ---- weighted frame mean + output assembly (single fused pass) ----
        wb_psum = psum_pool.tile([128, CB * T * S], _F32, tag="wb")
        for cb in range(CB):
            nc.tensor.matmul(wb_psum[:, cb * T * S:(cb + 1) * T * S],
                             lhsT=wrow[:], rhs=w[:], start=True, stop=True)
        prod = small_pool.tile([128, CB * T * S], _F32)
        nc.vector.scalar_tensor_tensor(
            prod[:], feats[:], WSCALE, wb_psum[:],
            op0=mybir.AluOpType.mult, op1=mybir.AluOpType.mult,
        )
        oblk = small_pool.tile([128, CB * S], _F32)
        nc.vector.tensor_reduce(
            oblk[:], prod[:].rearrange("p (b s t) -> p b s t", b=CB, s=S),
            axis=mybir.AxisListType.X, op=mybir.AluOpType.add,
        )
        tr_psum = psum_pool.tile([CB * S, 128], _F32, tag="tr")
        nc.tensor.transpose(tr_psum[:], oblk[:], identity[:])
        outt = small_pool.tile([CB * S, 128], _F32)
        nc.vector.tensor_copy(outt[:], tr_psum[:])
        nc.sync.dma_start(out.rearrange("s (b c) -> b s c", b=CB), outt[:])


_NC_CACHE = {}


def _get_nc(repeat=1):
    if repeat not in _NC_CACHE:
        nc = bacc.Bacc("TRN2", target_bir_lowering=False, debug=False)
        x = nc.dram_tensor("x", [C, T * FRAME], _F32, kind="ExternalInput").ap()
        out = nc.dram_tensor("out", [S, C], _F32, kind="ExternalOutput").ap()
        with tile.TileContext(nc) as tc:
            with ExitStack() as ctx:
                _kernel_body(ctx, tc, nc, x, out, repeat=repeat)
        nc.compile()
        _NC_CACHE[repeat] = nc
    return _NC_CACHE[repeat]


def kernel(x):
    x = np.asarray(x, dtype=np.float32)
    assert x.shape == (N, C, T, H, W), x.shape
    nc = _get_nc()
    in_maps = [{"x": np.ascontiguousarray(x[i]).reshape(C, T * FRAME)} for i in range(N)]
    res = run_bass_kernel_spmd(nc, in_maps, list(range(N)))
    return np.stack([res.results[i]["out"].reshape(S * C) for i in range(N)])



# revision 23
# speedup vs baseline: 1.0226x; 1.0226x over previous
"""Trainium2 Bass kernel for LocalSpatioTemporalPooling.

Reference computation (per sample n):
  x: (C=256, T=30, H=64, W=44) fp32
  feats[c,t,s] = mean over the (8,44) spatial stripe s of frame t    # 352-elem mean
  scores[t,s] = || feats[:,t,s] ||_2  (clip eps)                     # reduce over C
  top-2 frames per stripe by score; output[s*C + c] = mean of the 2 selected feats

Sharding: pure data parallel -- one sample per NeuronCore (N=8 = n_cores).

The DMA stream (86.5 MB/core) is the hard floor (~240.3us at 360 GB/s); the
kernel hides everything else behind it:
  - Frames 0..27 stream in interleaved cb0/cb1 4-frame ranges, then frame 28
    (both c-blocks), then frame 29 LAST in three pieces (cb0 full frame, cb1
    stripes 0-5, cb1 stripes 6-7).  Group sums are split across DVE, Pool
    (gpsimd) and ACT so no engine falls behind the stream.
  - While frame 28 streams: top-2 over frames 0..27 -> per-stripe max m1',
    second max m2', masks, and two dense weighted passes P = 0.5*mean(best),
    Q = 0.5*mean(second).
  - While frame 29 streams: frame 28 is merged in O(1) per stripe
    (top-2 of 0..28 = top-2 of {top-2 of 0..27} u {s28}):
    v0 = P + (s28>m2' ? h28 : Q), u = (s28>m1' ? h28 : P), m2'' updated;
    v0 is PE-transposed into the output buffer.
  - After the last byte only a short chain remains: 2-stripe sums, frame-29
    scores, per-stripe compare s29 > m2'', predicated copy of d = u + h29
    over v0, and the output DMA.
"""

import dataclasses

import numpy as np
from contextlib import ExitStack

import concourse.bass as bass
import concourse.tile as tile
import concourse.mybir as mybir
from concourse import bacc
from concourse.bass_utils import run_bass_kernel_spmd
from concourse.masks import make_identity

N, C, T, H, W = 8, 256, 30, 64, 44
S = 8                 # stripes
SH = H // S           # 8 rows per stripe
GROUP = SH * W        # 352 elements per (c, t, s) group
CB = C // 128         # 2 channel blocks
FRAME = H * W         # 2816
WSCALE = 0.5 / GROUP  # top-2 mean of stripe means
BIG = 1.0e30
TS = T * S            # 240 columns per c-block

# frames 0..27 in interleaved ranges (cb0 chunk then cb1 chunk each),
# tapering at the end so the speculative chain can start early
RANGES = [(0, 4), (4, 4), (8, 4), (12, 4), (16, 4), (20, 4), (24, 2), (26, 1), (27, 1)]
# DVE frame count per multi-frame chunk (cb, t0); ACT takes the rest
SPLIT_FR = {
    (0, 0): 2, (1, 0): 2, (0, 4): 2, (1, 4): 2, (0, 8): 2, (1, 8): 2,
    (0, 12): 2, (1, 12): 2, (0, 16): 2, (1, 16): 2, (0, 20): 2, (1, 20): 2,
    (0, 24): 1, (1, 24): 1,
}
# DVE stripe count for single-frame chunks (cb, t0); ACT takes the rest
SPLIT_ST = {(0, 26): 4, (1, 26): 4, (0, 27): 4, (1, 27): 6, (0, 28): 0, (1, 28): 6}
# engine computing the squares for each range ("a" = ACT, "v" = DVE)
SQ_ENGINE = {0: "a", 4: "a", 8: "a", 12: "a", 16: "a", 20: "a",
             24: "a", 26: "v", 27: "v"}
# frame-29 pieces: (cb, s0, sn, dve_stripes); ACT rest.
# cb-interleaved so sums can start while later pieces stream; the last piece
# is a single stripe reduced fully on DVE inside the tail chain.
F29_PIECES = [(0, 0, 4, 2), (1, 0, 4, 2), (0, 4, 4, 3), (1, 4, 3, 2), (1, 7, 1, 1)]

_F32 = mybir.dt.float32
_BF16 = mybir.dt.bfloat16


def _bcast(ap2d, inner):
    """(1, K) AP -> (1, K, inner) stride-0 broadcast view."""
    [pp, pc], [fs, fc] = ap2d.ap[0], ap2d.ap[1]
    return dataclasses.replace(ap2d, ap=[[pp, pc], [fs, fc], [0, inner]])


def _prow(ap2d, inner):
    """(P, 1) AP -> (P, inner) stride-0 broadcast along free."""
    [pp, pc] = ap2d.ap[0]
    return dataclasses.replace(ap2d, ap=[[pp, pc], [0, inner]])


def _cbrep(ap2d):
    """(1, 8) AP -> (1, 2, 8) view repeating the 8 free elems twice."""
    [pp, pc], [fs, fc] = ap2d.ap[0], ap2d.ap[1]
    return dataclasses.replace(ap2d, ap=[[pp, pc], [0, 2], [fs, fc]])


def _kernel_body(ctx, tc, nc, x, out, repeat=1):
    const_pool = ctx.enter_context(tc.tile_pool(name="const", bufs=1))
    in_pool = ctx.enter_context(tc.tile_pool(name="inp", bufs=3))
    f29_pool = ctx.enter_context(tc.tile_pool(name="f29", bufs=1))
    feat_pool = ctx.enter_context(tc.tile_pool(name="feat", bufs=1))
    small_pool = ctx.enter_context(tc.tile_pool(name="small", bufs=1))
    psum_pool = ctx.enter_context(tc.tile_pool(name="psum", bufs=1, space="PSUM"))

    ones_col = const_pool.tile([128, 1], _F32)
    nc.vector.memset(ones_col[:], 1.0)
    wrow = const_pool.tile([1, 128], _BF16)
    nc.vector.memset(wrow[:], 1.0)
    ones_row = const_pool.tile([1, 128], _F32)
    nc.vector.memset(ones_row[:], 1.0)
    identity = const_pool.tile([128, 128], _F32)
    make_identity(nc, identity[:])

    # [c-block 0 | c-block 1] side by side; free layout within a block: s*30 + t
    feats = feat_pool.tile([128, CB * TS], _F32)
    sq = feat_pool.tile([128, CB * TS], _F32)
    ss_psum = psum_pool.tile([1, TS], _F32, tag="ss")
    ssv = ss_psum[:].rearrange("p (s t) -> p s t", s=S)
    ssv27 = ssv[:, :, 0:T - 2]  # frames 0..27

    def fview(cb, t0, tc_, s0=0, sn=S):  # (128, tc_, sn) t-minor slice
        return feats[:, cb * TS:(cb + 1) * TS].rearrange(
            "p (s t) -> p t s", s=S)[:, t0:t0 + tc_, s0:s0 + sn]

    def fcols(t_abs, cb=None, s0=0, sn=None):
        # (128, k) stepped view of per-(cb,s) columns of frame t_abs
        if cb is None:
            return feats[:, t_abs:CB * TS:T]
        base = cb * TS
        sn = S - s0 if sn is None else sn
        return feats[:, base + s0 * T + t_abs:base + (s0 + sn) * T:T]

    def sqcols(t_abs, cb, s0=0, sn=None):
        base = cb * TS
        sn = S - s0 if sn is None else sn
        return sq[:, base + s0 * T + t_abs:base + (s0 + sn) * T:T]

    # scratch rows for ACT-side accumulate-copy (value discarded, accum kept)
    act_scratch = [feat_pool.tile([128, GROUP], _F32, name=f"actscr{i}")
                   for i in range(2)]
    act_n = [0]

    def act_group_sum(cb, tile_ap, g, t_abs, s_):
        # sum one (c, t, s) 352-group on the Scalar engine via accum_out
        scr = act_scratch[act_n[0] % 2]
        act_n[0] += 1
        col = cb * TS + s_ * T + t_abs
        nc.scalar.activation(
            scr[:], tile_ap[:, g * GROUP:(g + 1) * GROUP],
            mybir.ActivationFunctionType.Copy,
            accum_out=feats[:, col:col + 1],
        )

    def sview(t_, cb, t0, tc_):  # (128, 8, tc_) s-major slice
        return t_[:, cb * TS:(cb + 1) * TS].rearrange(
            "p (s t) -> p s t", s=S)[:, :, t0:t0 + tc_]

    def chunk_sums(cb, t0, tcn, tl, dve_fr):
        in4 = tl[:, :tcn * S * GROUP].rearrange(
            "p (t s w) -> p t s w", t=tcn, s=S)
        if dve_fr:
            nc.vector.tensor_reduce(
                fview(cb, t0, dve_fr), in4[:, :dve_fr, :, :],
                axis=mybir.AxisListType.X, op=mybir.AluOpType.add,
            )
        for tloc in range(dve_fr, tcn):
            for s_ in range(S):
                act_group_sum(cb, tl, tloc * S + s_, t0 + tloc, s_)

    def stripe_sums(cb, t_abs, tl, s_base, sn, kv):
        # single-frame piece: tl holds stripes [s_base, s_base+sn) of t_abs
        in3 = tl[:, :sn * GROUP].rearrange("p (s w) -> p s w", s=sn)
        if kv:
            nc.vector.tensor_reduce(
                fview(cb, t_abs, 1, s_base, kv), in3[:, 0:kv, :],
                axis=mybir.AxisListType.X, op=mybir.AluOpType.add,
            )
        for s_ in range(kv, sn):
            act_group_sum(cb, tl, s_, t_abs, s_base + s_)

    for _rep in range(repeat):
        # ---- frames 0..27: interleaved cb ranges ----
        for (t0, tcn) in RANGES:
            for cb in range(CB):
                tl = in_pool.tile([128, 4 * S * GROUP], _F32, name="tl", tag="tl")
                nc.sync.dma_start(
                    tl[:, :tcn * S * GROUP],
                    x[cb * 128:(cb + 1) * 128, t0 * FRAME:(t0 + tcn) * FRAME],
                )
                if tcn == 1:
                    stripe_sums(cb, t0, tl, 0, S, SPLIT_ST[(cb, t0)])
                else:
                    chunk_sums(cb, t0, tcn, tl, SPLIT_FR[(cb, t0)])
                if SQ_ENGINE[t0] == "a":
                    nc.scalar.activation(
                        sview(sq, cb, t0, tcn), sview(feats, cb, t0, tcn),
                        mybir.ActivationFunctionType.Square,
                    )
                else:
                    nc.vector.tensor_tensor(
                        sview(sq, cb, t0, tcn), sview(feats, cb, t0, tcn),
                        sview(feats, cb, t0, tcn), op=mybir.AluOpType.mult,
                    )
            for cb in range(CB):
                nc.tensor.matmul(
                    ssv[:, :, t0:t0 + tcn], lhsT=ones_col[:],
                    rhs=sview(sq, cb, t0, tcn),
                    start=(cb == 0), stop=(cb == 1),
                )

        # ---- speculative top-2 over frames 0..27 (overlaps frame-28 DMA) ----
        m1 = small_pool.tile([1, S], _F32)
        nc.vector.tensor_reduce(m1[:], ssv27, axis=mybir.AxisListType.X,
                                op=mybir.AluOpType.max)
        eq1 = small_pool.tile([1, TS], _F32)
        eq1v = eq1[:].rearrange("p (s t) -> p s t", s=S)[:, :, 0:T - 2]
        nc.vector.tensor_tensor(eq1v, ssv27, _bcast(m1[:], T - 2),
                                op=mybir.AluOpType.is_ge)
        masked = small_pool.tile([1, TS], _F32)
        maskedv = masked[:].rearrange("p (s t) -> p s t", s=S)[:, :, 0:T - 2]
        nc.vector.scalar_tensor_tensor(
            maskedv, eq1v, -BIG, ssv27,
            op0=mybir.AluOpType.mult, op1=mybir.AluOpType.add,
        )
        m2 = small_pool.tile([1, S], _F32)
        nc.vector.tensor_reduce(m2[:], maskedv, axis=mybir.AxisListType.X,
                                op=mybir.AluOpType.max)
        # masks over 0..27: w1 = best frame, wQ = second-best frame
        w1 = small_pool.tile([1, TS], _BF16)
        w1v = w1[:].rearrange("p (s t) -> p s t", s=S)[:, :, 0:T - 2]
        nc.vector.tensor_tensor(w1v, ssv27, _bcast(m1[:], T - 2),
                                op=mybir.AluOpType.is_ge)
        w2 = small_pool.tile([1, TS], _BF16)
        w2v = w2[:].rearrange("p (s t) -> p s t", s=S)[:, :, 0:T - 2]
        nc.vector.tensor_tensor(w2v, ssv27, _bcast(m2[:], T - 2),
                                op=mybir.AluOpType.is_ge)
        wQ = small_pool.tile([1, TS], _BF16)
        wQv = wQ[:].rearrange("p (s t) -> p s t", s=S)[:, :, 0:T - 2]
        nc.vector.tensor_tensor(wQv, w2v, w1v, op=mybir.AluOpType.subtract)

        wb1_psum = psum_pool.tile([128, CB * TS], _F32, tag="wb1")
        wbQ_psum = psum_pool.tile([128, CB * TS], _F32, tag="wbQ")
        for cb in range(CB):
            wb1s = sview(wb1_psum, cb, 0, T - 2)
            wbQs = sview(wbQ_psum, cb, 0, T - 2)
            nc.tensor.matmul(wb1s, lhsT=wrow[:], rhs=w1v, start=True, stop=True)
            nc.tensor.matmul(wbQs, lhsT=wrow[:], rhs=wQv, start=True, stop=True)

        def wpass(wb, dst):
            prod = small_pool.tile([128, CB * TS], _F32, tag="prod")
            pv = prod[:].rearrange("p (b s t) -> p b s t", b=CB, s=S)[:, :, :, 0:T - 2]
            fv = feats[:].rearrange("p (b s t) -> p b s t", b=CB, s=S)[:, :, :, 0:T - 2]
            wv = wb[:].rearrange("p (b s t) -> p b s t", b=CB, s=S)[:, :, :, 0:T - 2]
            nc.vector.scalar_tensor_tensor(
                pv, fv, WSCALE, wv,
                op0=mybir.AluOpType.mult, op1=mybir.AluOpType.mult,
            )
            nc.vector.tensor_reduce(
                dst[:].rearrange("p (b s) -> p b s", b=CB), pv,
                axis=mybir.AxisListType.X, op=mybir.AluOpType.add,
            )

        P = small_pool.tile([128, CB * S], _F32)
        Q = small_pool.tile([128, CB * S], _F32)
        wpass(wb1_psum, P)
        wpass(wbQ_psum, Q)

        # ---- frame 28 streams now; merge it into the candidates ----
        for cb in range(CB):
            tl = in_pool.tile([128, 4 * S * GROUP], _F32, name="tl", tag="tl")
            nc.sync.dma_start(
                tl[:, :S * GROUP],
                x[cb * 128:(cb + 1) * 128, (T - 2) * FRAME:(T - 1) * FRAME],
            )
            stripe_sums(cb, T - 2, tl, 0, S, SPLIT_ST[(cb, T - 2)])
            nc.vector.tensor_tensor(
                sqcols(T - 2, cb), fcols(T - 2, cb), fcols(T - 2, cb),
                op=mybir.AluOpType.mult,
            )
        for cb in range(CB):
            nc.tensor.matmul(
                ssv[:, :, T - 2:T - 1], lhsT=ones_col[:],
                rhs=sqcols(T - 2, cb).rearrange("p (x o) -> p x o", o=1),
                start=(cb == 0), stop=(cb == 1),
            )

        PQ = small_pool.tile([128, CB * S], _F32)
        nc.vector.tensor_tensor(PQ[:], P[:], Q[:], op=mybir.AluOpType.add)

        # (masks are exact 0.0/1.0 floats, so arithmetic blending is exact)
        s28 = ss_psum[:, T - 2:TS:T]  # (1, 8) stepped view
        h28 = small_pool.tile([128, CB * S], _F32)
        nc.vector.tensor_scalar_mul(h28[:], fcols(T - 2), WSCALE)
        hmQ = small_pool.tile([128, CB * S], _F32)
        nc.vector.tensor_tensor(hmQ[:], h28[:], Q[:], op=mybir.AluOpType.subtract)
        c28a = small_pool.tile([1, S], _F32)
        nc.vector.tensor_tensor(c28a[:], s28, m2[:], op=mybir.AluOpType.is_gt)
        c28b = small_pool.tile([1, S], _F32)
        nc.vector.tensor_tensor(c28b[:], s28, m1[:], op=mybir.AluOpType.is_gt)

        mask28 = psum_pool.tile([128, 2 * CB * S], _F32, tag="mAB")
        for cb in range(CB):
            nc.tensor.matmul(mask28[:, cb * S:(cb + 1) * S],
                             lhsT=ones_row[:], rhs=c28a[:],
                             start=True, stop=True, skip_group_check=True)
            nc.tensor.matmul(mask28[:, CB * S + cb * S:CB * S + (cb + 1) * S],
                             lhsT=ones_row[:], rhs=c28b[:],
                             start=True, stop=True, skip_group_check=True)

        # v0 = P + Q + mask_a*(h28 - Q);  u = P + mask_b*(h28 - P)
        t1 = small_pool.tile([128, CB * S], _F32)
        nc.vector.tensor_tensor(t1[:], mask28[:, 0:CB * S], hmQ[:],
                                op=mybir.AluOpType.mult)
        v0 = small_pool.tile([128, CB * S], _F32)
        nc.vector.tensor_tensor(v0[:], PQ[:], t1[:], op=mybir.AluOpType.add)

        # out_sb prefilled with v0^T: rows (cb, s), cols c
        trv0 = psum_pool.tile([CB * S, 128], _F32, tag="tr")
        nc.tensor.transpose(trv0[:], v0[:], identity[:])
        out_sb = small_pool.tile([CB * S, 128], _F32)
        nc.scalar.copy(out_sb[:], trv0[:])

        # off the critical path: u, uv = u - v0, merged second max
        hmP = small_pool.tile([128, CB * S], _F32)
        nc.vector.tensor_tensor(hmP[:], h28[:], P[:], op=mybir.AluOpType.subtract)
        t2 = small_pool.tile([128, CB * S], _F32)
        nc.vector.tensor_tensor(t2[:], mask28[:, CB * S:], hmP[:],
                                op=mybir.AluOpType.mult)
        u = small_pool.tile([128, CB * S], _F32)
        nc.vector.tensor_tensor(u[:], P[:], t2[:], op=mybir.AluOpType.add)
        uv = small_pool.tile([128, CB * S], _F32)
        nc.vector.tensor_tensor(uv[:], u[:], v0[:], op=mybir.AluOpType.subtract)
        n1 = small_pool.tile([1, S], _F32)
        nc.vector.tensor_tensor(n1[:], m1[:], s28, op=mybir.AluOpType.min)
        m2f = small_pool.tile([1, S], _F32)
        nc.vector.tensor_tensor(m2f[:], m2[:], n1[:], op=mybir.AluOpType.max)

        # ---- frame-29 pieces (DMAs overlap everything above) ----
        # per-stripe sumsq accumulates cb0 (start) + cb1 (stop) in PSUM
        ss29 = psum_pool.tile([1, S], _F32, tag="ss29")
        for pi, (cb, s0, sn, kv) in enumerate(F29_PIECES):
            tl = f29_pool.tile([128, sn * GROUP], _F32, name=f"f29_{pi}")
            nc.sync.dma_start(
                tl[:, :sn * GROUP],
                x[cb * 128:(cb + 1) * 128,
                  (T - 1) * FRAME + s0 * GROUP:(T - 1) * FRAME + (s0 + sn) * GROUP],
            )
            last = pi == len(F29_PIECES) - 1
            if last:
                stripe_sums(cb, T - 1, tl, s0, sn, sn)  # all on DVE, in-tail
                nc.vector.tensor_tensor(
                    sqcols(T - 1, cb, s0, sn), fcols(T - 1, cb, s0, sn),
                    fcols(T - 1, cb, s0, sn), op=mybir.AluOpType.mult,
                )
            else:
                stripe_sums(cb, T - 1, tl, s0, sn, kv)
                nc.scalar.activation(
                    sqcols(T - 1, cb, s0, sn), fcols(T - 1, cb, s0, sn),
                    mybir.ActivationFunctionType.Square,
                )
            nc.tensor.matmul(
                ss29[:, s0:s0 + sn], lhsT=ones_col[:],
                rhs=sqcols(T - 1, cb, s0, sn).rearrange("p (x o) -> p x o", o=1),
                start=(cb == 0), stop=(cb == 1), skip_group_check=True,
            )

        # e = (u - v0) + 0.5*mean(f29): the delta applied where frame 29 wins
        e = small_pool.tile([128, CB * S], _F32)
        nc.vector.scalar_tensor_tensor(
            e[:], fcols(T - 1), WSCALE, uv[:],
            op0=mybir.AluOpType.mult, op1=mybir.AluOpType.add,
        )
        tail_ps = psum_pool.tile([CB * S, 132], _F32, tag="tail")
        tre = tail_ps[:, 0:128]
        maskT = tail_ps[:, 128:129]
        nc.tensor.transpose(tre, e[:], identity[:])

        # per-stripe decision: does frame 29 beat the 0..28 second max?
        # (computed directly as a (1, 16) cb-replicated row)
        mask16 = small_pool.tile([1, CB * S], _F32)
        nc.vector.tensor_tensor(
            mask16[:].rearrange("p (b s) -> p b s", b=CB),
            _cbrep(ss29[:]), _cbrep(m2f[:]), op=mybir.AluOpType.is_gt)
        nc.tensor.matmul(maskT, lhsT=mask16[:], rhs=ones_col[0:1, :],
                         start=True, stop=True, skip_group_check=True)
        maskTs = small_pool.tile([CB * S, 1], _F32)
        nc.vector.tensor_copy(maskTs[:], maskT)

        # blend: out = v0^T + mask * e^T, then store
        t3 = small_pool.tile([CB * S, 128], _F32)
        nc.vector.tensor_tensor(t3[:], _prow(maskTs[:], 128), tre,
                                op=mybir.AluOpType.mult)
        nc.vector.tensor_tensor(out_sb[:], out_sb[:], t3[:],
                                op=mybir.AluOpType.add)
        nc.sync.dma_start(out.rearrange("s (b c) -> b s c", b=CB), out_sb[:])


_NC_CACHE = {}


def _get_nc(repeat=1):
    if repeat not in _NC_CACHE:
        nc = bacc.Bacc("TRN2", target_bir_lowering=False, debug=False)
        x = nc.dram_tensor("x", [C, T * FRAME], _F32, kind="ExternalInput").ap()
        out = nc.dram_tensor("out", [S, C], _F32, kind="ExternalOutput").ap()
        with tile.TileContext(nc) as tc:
            with ExitStack() as ctx:
                _kernel_body(ctx, tc, nc, x, out, repeat=repeat)
        nc.compile()
        _NC_CACHE[repeat] = nc
    return _NC_CACHE[repeat]


def kernel(x):
    x = np.asarray(x, dtype=np.float32)
    assert x.shape == (N, C, T, H, W), x.shape
    nc = _get_nc()
    in_maps = [{"x": np.ascontiguousarray(x[i]).reshape(C, T * FRAME)} for i in range(N)]
    res = run_bass_kernel_spmd(nc, in_maps, list(range(N)))
    return np.stack([res.results[i]["out"].reshape(S * C) for i in range(N)])


# revision 25
# speedup vs baseline: 1.0242x; 1.0016x over previous
"""Trainium2 Bass kernel for LocalSpatioTemporalPooling.

Reference computation (per sample n):
  x: (C=256, T=30, H=64, W=44) fp32
  feats[c,t,s] = mean over the (8,44) spatial stripe s of frame t    # 352-elem mean
  scores[t,s] = || feats[:,t,s] ||_2  (clip eps)                     # reduce over C
  top-2 frames per stripe by score; output[s*C + c] = mean of the 2 selected feats

Sharding: pure data parallel -- one sample per NeuronCore (N=8 = n_cores).

The DMA stream (86.5 MB/core) is the hard floor (~240.3us at 360 GB/s); the
kernel hides everything else behind it:
  - Frames 0..27 stream in interleaved cb0/cb1 4-frame ranges, then frame 28
    (both c-blocks), then frame 29 LAST in three pieces (cb0 full frame, cb1
    stripes 0-5, cb1 stripes 6-7).  Group sums are split across DVE, Pool
    (gpsimd) and ACT so no engine falls behind the stream.
  - While frame 28 streams: top-2 over frames 0..27 -> per-stripe max m1',
    second max m2', masks, and two dense weighted passes P = 0.5*mean(best),
    Q = 0.5*mean(second).
  - While frame 29 streams: frame 28 is merged in O(1) per stripe
    (top-2 of 0..28 = top-2 of {top-2 of 0..27} u {s28}):
    v0 = P + (s28>m2' ? h28 : Q), u = (s28>m1' ? h28 : P), m2'' updated;
    v0 is PE-transposed into the output buffer.
  - After the last byte only a short chain remains: 2-stripe sums, frame-29
    scores, per-stripe compare s29 > m2'', predicated copy of d = u + h29
    over v0, and the output DMA.
"""

import dataclasses

import numpy as np
from contextlib import ExitStack

import concourse.bass as bass
import concourse.tile as tile
import concourse.mybir as mybir
from concourse import bacc
from concourse.bass_utils import run_bass_kernel_spmd
from concourse.masks import make_identity

N, C, T, H, W = 8, 256, 30, 64, 44
S = 8                 # stripes
SH = H // S           # 8 rows per stripe
GROUP = SH * W        # 352 elements per (c, t, s) group
CB = C // 128         # 2 channel blocks
FRAME = H * W         # 2816
WSCALE = 0.5 / GROUP  # top-2 mean of stripe means
BIG = 1.0e30
TS = T * S            # 240 columns per c-block

# frames 0..27 in interleaved ranges (cb0 chunk then cb1 chunk each),
# tapering at the end so the speculative chain can start early
RANGES = [(0, 4), (4, 4), (8, 4), (12, 4), (16, 4), (20, 4), (24, 2), (26, 1), (27, 1)]
# DVE frame count per multi-frame chunk (cb, t0); ACT takes the rest
SPLIT_FR = {
    (0, 0): 2, (1, 0): 2, (0, 4): 2, (1, 4): 2, (0, 8): 2, (1, 8): 2,
    (0, 12): 2, (1, 12): 2, (0, 16): 2, (1, 16): 2, (0, 20): 2, (1, 20): 2,
    (0, 24): 1, (1, 24): 1,
}
# DVE stripe count for single-frame chunks (cb, t0); ACT takes the rest
SPLIT_ST = {(0, 26): 4, (1, 26): 4, (0, 27): 4, (1, 27): 6, (0, 28): 0, (1, 28): 6}
# engine computing the squares for each range ("a" = ACT, "v" = DVE)
SQ_ENGINE = {0: "a", 4: "a", 8: "a", 12: "a", 16: "a", 20: "a",
             24: "a", 26: "v", 27: "v"}
# frame-29 pieces: (cb, s0, sn, dve_stripes); ACT rest.
# cb-interleaved so sums can start while later pieces stream; the last piece
# is a single stripe reduced fully on DVE inside the tail chain.
F29_PIECES = [(0, 0, 4, 2), (1, 0, 4, 2), (0, 4, 4, 3), (1, 4, 3, 2), (1, 7, 1, 1)]

_F32 = mybir.dt.float32
_BF16 = mybir.dt.bfloat16


def _bcast(ap2d, inner):
    """(1, K) AP -> (1, K, inner) stride-0 broadcast view."""
    [pp, pc], [fs, fc] = ap2d.ap[0], ap2d.ap[1]
    return dataclasses.replace(ap2d, ap=[[pp, pc], [fs, fc], [0, inner]])


def _prow(ap2d, inner):
    """(P, 1) AP -> (P, inner) stride-0 broadcast along free."""
    [pp, pc] = ap2d.ap[0]
    return dataclasses.replace(ap2d, ap=[[pp, pc], [0, inner]])


def _cbrep(ap2d):
    """(1, 8) AP -> (1, 2, 8) view repeating the 8 free elems twice."""
    [pp, pc], [fs, fc] = ap2d.ap[0], ap2d.ap[1]
    return dataclasses.replace(ap2d, ap=[[pp, pc], [0, 2], [fs, fc]])


def _kernel_body(ctx, tc, nc, x, out, repeat=1):
    const_pool = ctx.enter_context(tc.tile_pool(name="const", bufs=1))
    in_pool = ctx.enter_context(tc.tile_pool(name="inp", bufs=3))
    f29_pool = ctx.enter_context(tc.tile_pool(name="f29", bufs=1))
    feat_pool = ctx.enter_context(tc.tile_pool(name="feat", bufs=1))
    small_pool = ctx.enter_context(tc.tile_pool(name="small", bufs=1))
    psum_pool = ctx.enter_context(tc.tile_pool(name="psum", bufs=1, space="PSUM"))

    ones_col = const_pool.tile([128, 1], _F32)
    nc.vector.memset(ones_col[:], 1.0)
    wrow = const_pool.tile([1, 128], _BF16)
    nc.vector.memset(wrow[:], 1.0)
    ones_row = const_pool.tile([1, 128], _F32)
    nc.vector.memset(ones_row[:], 1.0)
    identity = const_pool.tile([128, 128], _F32)
    make_identity(nc, identity[:])

    # [c-block 0 | c-block 1] side by side; free layout within a block: s*30 + t
    feats = feat_pool.tile([128, CB * TS], _F32)
    sq = feat_pool.tile([128, CB * TS], _F32)
    ss_psum = psum_pool.tile([1, TS], _F32, tag="ss")
    ssv = ss_psum[:].rearrange("p (s t) -> p s t", s=S)
    ssv27 = ssv[:, :, 0:T - 2]  # frames 0..27

    def fview(cb, t0, tc_, s0=0, sn=S):  # (128, tc_, sn) t-minor slice
        return feats[:, cb * TS:(cb + 1) * TS].rearrange(
            "p (s t) -> p t s", s=S)[:, t0:t0 + tc_, s0:s0 + sn]

    def fcols(t_abs, cb=None, s0=0, sn=None):
        # (128, k) stepped view of per-(cb,s) columns of frame t_abs
        if cb is None:
            return feats[:, t_abs:CB * TS:T]
        base = cb * TS
        sn = S - s0 if sn is None else sn
        return feats[:, base + s0 * T + t_abs:base + (s0 + sn) * T:T]

    def sqcols(t_abs, cb, s0=0, sn=None):
        base = cb * TS
        sn = S - s0 if sn is None else sn
        return sq[:, base + s0 * T + t_abs:base + (s0 + sn) * T:T]

    # scratch rows for ACT-side accumulate-copy (value discarded, accum kept)
    act_scratch = [feat_pool.tile([128, GROUP], _F32, name=f"actscr{i}")
                   for i in range(2)]
    act_n = [0]

    def act_group_sum(cb, tile_ap, g, t_abs, s_):
        # sum one (c, t, s) 352-group on the Scalar engine via accum_out
        scr = act_scratch[act_n[0] % 2]
        act_n[0] += 1
        col = cb * TS + s_ * T + t_abs
        nc.scalar.activation(
            scr[:], tile_ap[:, g * GROUP:(g + 1) * GROUP],
            mybir.ActivationFunctionType.Copy,
            accum_out=feats[:, col:col + 1],
        )

    def sview(t_, cb, t0, tc_):  # (128, 8, tc_) s-major slice
        return t_[:, cb * TS:(cb + 1) * TS].rearrange(
            "p (s t) -> p s t", s=S)[:, :, t0:t0 + tc_]

    def chunk_sums(cb, t0, tcn, tl, dve_fr):
        in4 = tl[:, :tcn * S * GROUP].rearrange(
            "p (t s w) -> p t s w", t=tcn, s=S)
        if dve_fr:
            nc.vector.tensor_reduce(
                fview(cb, t0, dve_fr), in4[:, :dve_fr, :, :],
                axis=mybir.AxisListType.X, op=mybir.AluOpType.add,
            )
        for tloc in range(dve_fr, tcn):
            for s_ in range(S):
                act_group_sum(cb, tl, tloc * S + s_, t0 + tloc, s_)

    def stripe_sums(cb, t_abs, tl, s_base, sn, kv):
        # single-frame piece: tl holds stripes [s_base, s_base+sn) of t_abs
        in3 = tl[:, :sn * GROUP].rearrange("p (s w) -> p s w", s=sn)
        if kv:
            nc.vector.tensor_reduce(
                fview(cb, t_abs, 1, s_base, kv), in3[:, 0:kv, :],
                axis=mybir.AxisListType.X, op=mybir.AluOpType.add,
            )
        for s_ in range(kv, sn):
            act_group_sum(cb, tl, s_, t_abs, s_base + s_)

    for _rep in range(repeat):
        # ---- frames 0..27: interleaved cb ranges ----
        for (t0, tcn) in RANGES:
            for cb in range(CB):
                tl = in_pool.tile([128, 4 * S * GROUP], _F32, name="tl", tag="tl")
                nc.sync.dma_start(
                    tl[:, :tcn * S * GROUP],
                    x[cb * 128:(cb + 1) * 128, t0 * FRAME:(t0 + tcn) * FRAME],
                )
                if tcn == 1:
                    stripe_sums(cb, t0, tl, 0, S, SPLIT_ST[(cb, t0)])
                else:
                    chunk_sums(cb, t0, tcn, tl, SPLIT_FR[(cb, t0)])
                if SQ_ENGINE[t0] == "a":
                    nc.scalar.activation(
                        sview(sq, cb, t0, tcn), sview(feats, cb, t0, tcn),
                        mybir.ActivationFunctionType.Square,
                    )
                else:
                    nc.vector.tensor_tensor(
                        sview(sq, cb, t0, tcn), sview(feats, cb, t0, tcn),
                        sview(feats, cb, t0, tcn), op=mybir.AluOpType.mult,
                    )
            for cb in range(CB):
                nc.tensor.matmul(
                    ssv[:, :, t0:t0 + tcn], lhsT=ones_col[:],
                    rhs=sview(sq, cb, t0, tcn),
                    start=(cb == 0), stop=(cb == 1),
                )

        # ---- speculative top-2 over frames 0..27 (overlaps frame-28 DMA) ----
        m1 = small_pool.tile([1, S], _F32)
        nc.vector.tensor_reduce(m1[:], ssv27, axis=mybir.AxisListType.X,
                                op=mybir.AluOpType.max)
        eq1 = small_pool.tile([1, TS], _F32)
        eq1v = eq1[:].rearrange("p (s t) -> p s t", s=S)[:, :, 0:T - 2]
        nc.vector.tensor_tensor(eq1v, ssv27, _bcast(m1[:], T - 2),
                                op=mybir.AluOpType.is_ge)
        masked = small_pool.tile([1, TS], _F32)
        maskedv = masked[:].rearrange("p (s t) -> p s t", s=S)[:, :, 0:T - 2]
        nc.vector.scalar_tensor_tensor(
            maskedv, eq1v, -BIG, ssv27,
            op0=mybir.AluOpType.mult, op1=mybir.AluOpType.add,
        )
        m2 = small_pool.tile([1, S], _F32)
        nc.vector.tensor_reduce(m2[:], maskedv, axis=mybir.AxisListType.X,
                                op=mybir.AluOpType.max)
        # masks over 0..27: w1 = best frame, wQ = second-best frame
        w1 = small_pool.tile([1, TS], _BF16)
        w1v = w1[:].rearrange("p (s t) -> p s t", s=S)[:, :, 0:T - 2]
        nc.vector.tensor_tensor(w1v, ssv27, _bcast(m1[:], T - 2),
                                op=mybir.AluOpType.is_ge)
        w2 = small_pool.tile([1, TS], _BF16)
        w2v = w2[:].rearrange("p (s t) -> p s t", s=S)[:, :, 0:T - 2]
        nc.vector.tensor_tensor(w2v, ssv27, _bcast(m2[:], T - 2),
                                op=mybir.AluOpType.is_ge)
        wQ = small_pool.tile([1, TS], _BF16)
        wQv = wQ[:].rearrange("p (s t) -> p s t", s=S)[:, :, 0:T - 2]
        nc.vector.tensor_tensor(wQv, w2v, w1v, op=mybir.AluOpType.subtract)

        wb1_psum = psum_pool.tile([128, CB * TS], _F32, tag="wb1")
        wbQ_psum = psum_pool.tile([128, CB * TS], _F32, tag="wbQ")
        for cb in range(CB):
            wb1s = sview(wb1_psum, cb, 0, T - 2)
            wbQs = sview(wbQ_psum, cb, 0, T - 2)
            nc.tensor.matmul(wb1s, lhsT=wrow[:], rhs=w1v, start=True, stop=True)
            nc.tensor.matmul(wbQs, lhsT=wrow[:], rhs=wQv, start=True, stop=True)

        def wpass(wb, dst):
            prod = small_pool.tile([128, CB * TS], _F32, tag="prod")
            pv = prod[:].rearrange("p (b s t) -> p b s t", b=CB, s=S)[:, :, :, 0:T - 2]
            fv = feats[:].rearrange("p (b s t) -> p b s t", b=CB, s=S)[:, :, :, 0:T - 2]
            wv = wb[:].rearrange("p (b s t) -> p b s t", b=CB, s=S)[:, :, :, 0:T - 2]
            nc.vector.scalar_tensor_tensor(
                pv, fv, WSCALE, wv,
                op0=mybir.AluOpType.mult, op1=mybir.AluOpType.mult,
            )
            nc.vector.tensor_reduce(
                dst[:].rearrange("p (b s) -> p b s", b=CB), pv,
                axis=mybir.AxisListType.X, op=mybir.AluOpType.add,
            )

        P = small_pool.tile([128, CB * S], _F32)
        Q = small_pool.tile([128, CB * S], _F32)
        wpass(wb1_psum, P)
        wpass(wbQ_psum, Q)

        # ---- frame 28 streams now; merge it into the candidates ----
        for cb in range(CB):
            tl = in_pool.tile([128, 4 * S * GROUP], _F32, name="tl", tag="tl")
            nc.sync.dma_start(
                tl[:, :S * GROUP],
                x[cb * 128:(cb + 1) * 128, (T - 2) * FRAME:(T - 1) * FRAME],
            )
            stripe_sums(cb, T - 2, tl, 0, S, SPLIT_ST[(cb, T - 2)])
            nc.vector.tensor_tensor(
                sqcols(T - 2, cb), fcols(T - 2, cb), fcols(T - 2, cb),
                op=mybir.AluOpType.mult,
            )
        for cb in range(CB):
            nc.tensor.matmul(
                ssv[:, :, T - 2:T - 1], lhsT=ones_col[:],
                rhs=sqcols(T - 2, cb).rearrange("p (x o) -> p x o", o=1),
                start=(cb == 0), stop=(cb == 1),
            )

        PQ = small_pool.tile([128, CB * S], _F32)
        nc.vector.tensor_tensor(PQ[:], P[:], Q[:], op=mybir.AluOpType.add)

        # (masks are exact 0.0/1.0 floats, so arithmetic blending is exact)
        s28 = ss_psum[:, T - 2:TS:T]  # (1, 8) stepped view
        h28 = small_pool.tile([128, CB * S], _F32)
        nc.vector.tensor_scalar_mul(h28[:], fcols(T - 2), WSCALE)
        hmQ = small_pool.tile([128, CB * S], _F32)
        nc.vector.tensor_tensor(hmQ[:], h28[:], Q[:], op=mybir.AluOpType.subtract)
        c28a = small_pool.tile([1, S], _F32)
        nc.vector.tensor_tensor(c28a[:], s28, m2[:], op=mybir.AluOpType.is_gt)
        c28b = small_pool.tile([1, S], _F32)
        nc.vector.tensor_tensor(c28b[:], s28, m1[:], op=mybir.AluOpType.is_gt)

        mask28 = psum_pool.tile([128, 2 * CB * S], _F32, tag="mAB")
        for cb in range(CB):
            nc.tensor.matmul(mask28[:, cb * S:(cb + 1) * S],
                             lhsT=ones_row[:], rhs=c28a[:],
                             start=True, stop=True, skip_group_check=True)
            nc.tensor.matmul(mask28[:, CB * S + cb * S:CB * S + (cb + 1) * S],
                             lhsT=ones_row[:], rhs=c28b[:],
                             start=True, stop=True, skip_group_check=True)

        # v0 = P + Q + mask_a*(h28 - Q);  u = P + mask_b*(h28 - P)
        t1 = small_pool.tile([128, CB * S], _F32)
        nc.vector.tensor_tensor(t1[:], mask28[:, 0:CB * S], hmQ[:],
                                op=mybir.AluOpType.mult)
        v0 = small_pool.tile([128, CB * S], _F32)
        nc.vector.tensor_tensor(v0[:], PQ[:], t1[:], op=mybir.AluOpType.add)

        # out_sb prefilled with v0^T: rows (cb, s), cols c
        trv0 = psum_pool.tile([CB * S, 128], _F32, tag="tr")
        nc.tensor.transpose(trv0[:], v0[:], identity[:])
        out_sb = small_pool.tile([CB * S, 128], _F32)
        nc.scalar.copy(out_sb[:], trv0[:])

        # off the critical path: u, uv = u - v0, merged second max
        hmP = small_pool.tile([128, CB * S], _F32)
        nc.vector.tensor_tensor(hmP[:], h28[:], P[:], op=mybir.AluOpType.subtract)
        t2 = small_pool.tile([128, CB * S], _F32)
        nc.vector.tensor_tensor(t2[:], mask28[:, CB * S:], hmP[:],
                                op=mybir.AluOpType.mult)
        u = small_pool.tile([128, CB * S], _F32)
        nc.vector.tensor_tensor(u[:], P[:], t2[:], op=mybir.AluOpType.add)
        uv = small_pool.tile([128, CB * S], _F32)
        nc.vector.tensor_tensor(uv[:], u[:], v0[:], op=mybir.AluOpType.subtract)
        n1 = small_pool.tile([1, S], _F32)
        nc.vector.tensor_tensor(n1[:], m1[:], s28, op=mybir.AluOpType.min)
        m2f = small_pool.tile([1, S], _F32)
        nc.vector.tensor_tensor(m2f[:], m2[:], n1[:], op=mybir.AluOpType.max)

        # ---- frame-29 pieces (DMAs overlap everything above) ----
        # per-stripe sumsq accumulates cb0 (start) + cb1 (stop) in PSUM
        ss29 = psum_pool.tile([1, S], _F32, tag="ss29")
        for pi, (cb, s0, sn, kv) in enumerate(F29_PIECES):
            tl = f29_pool.tile([128, sn * GROUP], _F32, name=f"f29_{pi}")
            nc.sync.dma_start(
                tl[:, :sn * GROUP],
                x[cb * 128:(cb + 1) * 128,
                  (T - 1) * FRAME + s0 * GROUP:(T - 1) * FRAME + (s0 + sn) * GROUP],
            )
            last = pi == len(F29_PIECES) - 1
            if last:
                stripe_sums(cb, T - 1, tl, s0, sn, sn)  # all on DVE, in-tail
                nc.vector.tensor_tensor(
                    sqcols(T - 1, cb, s0, sn), fcols(T - 1, cb, s0, sn),
                    fcols(T - 1, cb, s0, sn), op=mybir.AluOpType.mult,
                )
            else:
                stripe_sums(cb, T - 1, tl, s0, sn, kv)
                nc.gpsimd.tensor_tensor(
                    sqcols(T - 1, cb, s0, sn), fcols(T - 1, cb, s0, sn),
                    fcols(T - 1, cb, s0, sn), op=mybir.AluOpType.mult,
                )
            nc.tensor.matmul(
                ss29[:, s0:s0 + sn], lhsT=ones_col[:],
                rhs=sqcols(T - 1, cb, s0, sn).rearrange("p (x o) -> p x o", o=1),
                start=(cb == 0), stop=(cb == 1), skip_group_check=True,
            )

        # e = (u - v0) + 0.5*mean(f29): the delta applied where frame 29 wins
        e = small_pool.tile([128, CB * S], _F32)
        nc.vector.scalar_tensor_tensor(
            e[:], fcols(T - 1), WSCALE, uv[:],
            op0=mybir.AluOpType.mult, op1=mybir.AluOpType.add,
        )
        tail_ps = psum_pool.tile([CB * S, 132], _F32, tag="tail")
        tre = tail_ps[:, 0:128]
        maskT = tail_ps[:, 128:129]
        nc.tensor.transpose(tre, e[:], identity[:])

        # per-stripe decision: does frame 29 beat the 0..28 second max?
        # (computed directly as a (1, 16) cb-replicated row)
        mask16 = small_pool.tile([1, CB * S], _F32)
        nc.vector.tensor_tensor(
            mask16[:].rearrange("p (b s) -> p b s", b=CB),
            _cbrep(ss29[:]), _cbrep(m2f[:]), op=mybir.AluOpType.is_gt)
        nc.tensor.matmul(maskT, lhsT=mask16[:], rhs=ones_col[0:1, :],
                         start=True, stop=True, skip_group_check=True)
        maskTs = small_pool.tile([CB * S, 1], _F32)
        nc.vector.tensor_copy(maskTs[:], maskT)

        # blend: out = v0^T + mask * e^T (single fused op), then store
        nc.vector.scalar_tensor_tensor(
            out_sb[:], tre, maskTs[:], out_sb[:],
            op0=mybir.AluOpType.mult, op1=mybir.AluOpType.add,
        )
        nc.sync.dma_start(out.rearrange("s (b c) -> b s c", b=CB), out_sb[:])


_NC_CACHE = {}


def _get_nc(repeat=1):
    if repeat not in _NC_CACHE:
        nc = bacc.Bacc("TRN2", target_bir_lowering=False, debug=False)
        x = nc.dram_tensor("x", [C, T * FRAME], _F32, kind="ExternalInput").ap()
        out = nc.dram_tensor("out", [S, C], _F32, kind="ExternalOutput").ap()
        with tile.TileContext(nc) as tc:
            with ExitStack() as ctx:
                _kernel_body(ctx, tc, nc, x, out, repeat=repeat)
        nc.compile()
        _NC_CACHE[repeat] = nc
    return _NC_CACHE[repeat]


def kernel(x):
    x = np.asarray(x, dtype=np.float32)
    assert x.shape == (N, C, T, H, W), x.shape
    nc = _get_nc()
    in_maps = [{"x": np.ascontiguousarray(x[i]).reshape(C, T * FRAME)} for i in range(N)]
    res = run_bass_kernel_spmd(nc, in_maps, list(range(N)))
    return np.stack([res.results[i]["out"].reshape(S * C) for i in range(N)])


# revision 37
# speedup vs baseline: 1.0249x; 1.0007x over previous
"""Trainium2 Bass kernel for LocalSpatioTemporalPooling.

Reference computation (per sample n):
  x: (C=256, T=30, H=64, W=44) fp32
  feats[c,t,s] = mean over the (8,44) spatial stripe s of frame t    # 352-elem mean
  scores[t,s] = || feats[:,t,s] ||_2  (clip eps)                     # reduce over C
  top-2 frames per stripe by score; output[s*C + c] = mean of the 2 selected feats

Sharding: pure data parallel -- one sample per NeuronCore (N=8 = n_cores).

The DMA stream (86.5 MB/core) is the hard floor (~240.3us at 360 GB/s); the
kernel hides everything else behind it:
  - Frames 0..27 stream in interleaved cb0/cb1 4-frame ranges, then frame 28
    (both c-blocks), then frame 29 LAST in three pieces (cb0 full frame, cb1
    stripes 0-5, cb1 stripes 6-7).  Group sums are split across DVE, Pool
    (gpsimd) and ACT so no engine falls behind the stream.
  - While frame 28 streams: top-2 over frames 0..27 -> per-stripe max m1',
    second max m2', masks, and two dense weighted passes P = 0.5*mean(best),
    Q = 0.5*mean(second).
  - While frame 29 streams: frame 28 is merged in O(1) per stripe
    (top-2 of 0..28 = top-2 of {top-2 of 0..27} u {s28}):
    v0 = P + (s28>m2' ? h28 : Q), u = (s28>m1' ? h28 : P), m2'' updated;
    v0 is PE-transposed into the output buffer.
  - After the last byte only a short chain remains: 2-stripe sums, frame-29
    scores, per-stripe compare s29 > m2'', predicated copy of d = u + h29
    over v0, and the output DMA.
"""

import dataclasses

import numpy as np
from contextlib import ExitStack

import concourse.bass as bass
import concourse.tile as tile
import concourse.mybir as mybir
from concourse import bacc
from concourse.bass_utils import run_bass_kernel_spmd
from concourse.masks import make_identity

N, C, T, H, W = 8, 256, 30, 64, 44
S = 8                 # stripes
SH = H // S           # 8 rows per stripe
GROUP = SH * W        # 352 elements per (c, t, s) group
CB = C // 128         # 2 channel blocks
FRAME = H * W         # 2816
WSCALE = 0.5 / GROUP  # top-2 mean of stripe means
BIG = 1.0e30
TS = T * S            # 240 columns per c-block

# frames 0..27 in interleaved ranges (cb0 chunk then cb1 chunk each),
# tapering at the end so the speculative chain can start early
RANGES = [(0, 4), (4, 4), (8, 4), (12, 4), (16, 4), (20, 4), (24, 2), (26, 1), (27, 1)]
# DVE frame count per multi-frame chunk (cb, t0); ACT takes the rest
SPLIT_FR = {
    (0, 0): 2, (1, 0): 2, (0, 4): 2, (1, 4): 2, (0, 8): 2, (1, 8): 2,
    (0, 12): 2, (1, 12): 2, (0, 16): 2, (1, 16): 2, (0, 20): 2, (1, 20): 2,
    (0, 24): 1, (1, 24): 1,
}
# DVE stripe count for single-frame chunks (cb, t0); ACT takes the rest
SPLIT_ST = {(0, 26): 4, (1, 26): 4, (0, 27): 4, (1, 27): 6, (0, 28): 0, (1, 28): 6}
# engine computing the squares for each range ("a" = ACT, "v" = DVE)
SQ_ENGINE = {0: "a", 4: "a", 8: "a", 12: "a", 16: "a", 20: "a",
             24: "a", 26: "v", 27: "v"}
# frame-29 pieces: (cb, s0, sn, dve_stripes); ACT rest.
# cb-interleaved so sums can start while later pieces stream; the last piece
# is a single stripe reduced fully on DVE inside the tail chain.
F29_PIECES = [(0, 0, 4, 2), (1, 0, 4, 2), (0, 4, 4, 2), (1, 4, 3, 2), (1, 7, 1, 1)]

_F32 = mybir.dt.float32
_BF16 = mybir.dt.bfloat16


def _bcast(ap2d, inner):
    """(1, K) AP -> (1, K, inner) stride-0 broadcast view."""
    [pp, pc], [fs, fc] = ap2d.ap[0], ap2d.ap[1]
    return dataclasses.replace(ap2d, ap=[[pp, pc], [fs, fc], [0, inner]])


def _prow(ap2d, inner):
    """(P, 1) AP -> (P, inner) stride-0 broadcast along free."""
    [pp, pc] = ap2d.ap[0]
    return dataclasses.replace(ap2d, ap=[[pp, pc], [0, inner]])


def _cbrep(ap2d):
    """(1, 8) AP -> (1, 2, 8) view repeating the 8 free elems twice."""
    [pp, pc], [fs, fc] = ap2d.ap[0], ap2d.ap[1]
    return dataclasses.replace(ap2d, ap=[[pp, pc], [0, 2], [fs, fc]])


def _kernel_body(ctx, tc, nc, x, out, repeat=1):
    const_pool = ctx.enter_context(tc.tile_pool(name="const", bufs=1))
    in_pool = ctx.enter_context(tc.tile_pool(name="inp", bufs=3))
    f29_pool = ctx.enter_context(tc.tile_pool(name="f29", bufs=1))
    feat_pool = ctx.enter_context(tc.tile_pool(name="feat", bufs=1))
    small_pool = ctx.enter_context(tc.tile_pool(name="small", bufs=1))
    psum_pool = ctx.enter_context(tc.tile_pool(name="psum", bufs=1, space="PSUM"))

    ones_col = const_pool.tile([128, 1], _F32)
    nc.vector.memset(ones_col[:], 1.0)
    wrow = const_pool.tile([1, 128], _BF16)
    nc.vector.memset(wrow[:], 1.0)
    ones_row = const_pool.tile([1, 128], _F32)
    nc.vector.memset(ones_row[:], 1.0)
    identity = const_pool.tile([128, 128], _F32)
    make_identity(nc, identity[:])

    # [c-block 0 | c-block 1] side by side; free layout within a block: s*30 + t
    feats = feat_pool.tile([128, CB * TS], _F32)
    sq = feat_pool.tile([128, CB * TS], _F32)
    ss_psum = psum_pool.tile([1, TS], _F32, tag="ss")
    ssv = ss_psum[:].rearrange("p (s t) -> p s t", s=S)
    ssv27 = ssv[:, :, 0:T - 2]  # frames 0..27

    def fview(cb, t0, tc_, s0=0, sn=S):  # (128, tc_, sn) t-minor slice
        return feats[:, cb * TS:(cb + 1) * TS].rearrange(
            "p (s t) -> p t s", s=S)[:, t0:t0 + tc_, s0:s0 + sn]

    def fcols(t_abs, cb=None, s0=0, sn=None):
        # (128, k) stepped view of per-(cb,s) columns of frame t_abs
        if cb is None:
            return feats[:, t_abs:CB * TS:T]
        base = cb * TS
        sn = S - s0 if sn is None else sn
        return feats[:, base + s0 * T + t_abs:base + (s0 + sn) * T:T]

    def sqcols(t_abs, cb, s0=0, sn=None):
        base = cb * TS
        sn = S - s0 if sn is None else sn
        return sq[:, base + s0 * T + t_abs:base + (s0 + sn) * T:T]

    # scratch rows for ACT-side accumulate-copy (value discarded, accum kept)
    act_scratch = [feat_pool.tile([128, GROUP], _F32, name=f"actscr{i}")
                   for i in range(2)]
    act_n = [0]

    def act_group_sum(cb, tile_ap, g, t_abs, s_):
        # sum one (c, t, s) 352-group on the Scalar engine via accum_out
        scr = act_scratch[act_n[0] % 2]
        act_n[0] += 1
        col = cb * TS + s_ * T + t_abs
        nc.scalar.activation(
            scr[:], tile_ap[:, g * GROUP:(g + 1) * GROUP],
            mybir.ActivationFunctionType.Copy,
            accum_out=feats[:, col:col + 1],
        )

    def sview(t_, cb, t0, tc_):  # (128, 8, tc_) s-major slice
        return t_[:, cb * TS:(cb + 1) * TS].rearrange(
            "p (s t) -> p s t", s=S)[:, :, t0:t0 + tc_]

    def chunk_sums(cb, t0, tcn, tl, dve_fr):
        in4 = tl[:, :tcn * S * GROUP].rearrange(
            "p (t s w) -> p t s w", t=tcn, s=S)
        if dve_fr:
            nc.vector.tensor_reduce(
                fview(cb, t0, dve_fr), in4[:, :dve_fr, :, :],
                axis=mybir.AxisListType.X, op=mybir.AluOpType.add,
            )
        for tloc in range(dve_fr, tcn):
            for s_ in range(S):
                act_group_sum(cb, tl, tloc * S + s_, t0 + tloc, s_)

    def stripe_sums(cb, t_abs, tl, s_base, sn, kv):
        # single-frame piece: tl holds stripes [s_base, s_base+sn) of t_abs
        in3 = tl[:, :sn * GROUP].rearrange("p (s w) -> p s w", s=sn)
        if kv:
            nc.vector.tensor_reduce(
                fview(cb, t_abs, 1, s_base, kv), in3[:, 0:kv, :],
                axis=mybir.AxisListType.X, op=mybir.AluOpType.add,
            )
        for s_ in range(kv, sn):
            act_group_sum(cb, tl, s_, t_abs, s_base + s_)

    for _rep in range(repeat):
        # ---- frames 0..27: interleaved cb ranges ----
        for (t0, tcn) in RANGES:
            for cb in range(CB):
                tl = in_pool.tile([128, 4 * S * GROUP], _F32, name="tl", tag="tl")
                nc.sync.dma_start(
                    tl[:, :tcn * S * GROUP],
                    x[cb * 128:(cb + 1) * 128, t0 * FRAME:(t0 + tcn) * FRAME],
                )
                if tcn == 1:
                    stripe_sums(cb, t0, tl, 0, S, SPLIT_ST[(cb, t0)])
                else:
                    chunk_sums(cb, t0, tcn, tl, SPLIT_FR[(cb, t0)])
                if SQ_ENGINE[t0] == "a":
                    nc.scalar.activation(
                        sview(sq, cb, t0, tcn), sview(feats, cb, t0, tcn),
                        mybir.ActivationFunctionType.Square,
                    )
                else:
                    nc.vector.tensor_tensor(
                        sview(sq, cb, t0, tcn), sview(feats, cb, t0, tcn),
                        sview(feats, cb, t0, tcn), op=mybir.AluOpType.mult,
                    )
            for cb in range(CB):
                nc.tensor.matmul(
                    ssv[:, :, t0:t0 + tcn], lhsT=ones_col[:],
                    rhs=sview(sq, cb, t0, tcn),
                    start=(cb == 0), stop=(cb == 1),
                )

        # ---- speculative top-2 over frames 0..27 (overlaps frame-28 DMA) ----
        m1 = small_pool.tile([1, S], _F32)
        nc.vector.tensor_reduce(m1[:], ssv27, axis=mybir.AxisListType.X,
                                op=mybir.AluOpType.max)
        eq1 = small_pool.tile([1, TS], _F32)
        eq1v = eq1[:].rearrange("p (s t) -> p s t", s=S)[:, :, 0:T - 2]
        nc.vector.tensor_tensor(eq1v, ssv27, _bcast(m1[:], T - 2),
                                op=mybir.AluOpType.is_ge)
        masked = small_pool.tile([1, TS], _F32)
        maskedv = masked[:].rearrange("p (s t) -> p s t", s=S)[:, :, 0:T - 2]
        nc.vector.scalar_tensor_tensor(
            maskedv, eq1v, -BIG, ssv27,
            op0=mybir.AluOpType.mult, op1=mybir.AluOpType.add,
        )
        m2 = small_pool.tile([1, S], _F32)
        nc.vector.tensor_reduce(m2[:], maskedv, axis=mybir.AxisListType.X,
                                op=mybir.AluOpType.max)
        # masks over 0..27: w1 = best frame, wQ = second-best frame
        w1 = small_pool.tile([1, TS], _BF16)
        w1v = w1[:].rearrange("p (s t) -> p s t", s=S)[:, :, 0:T - 2]
        nc.vector.tensor_tensor(w1v, ssv27, _bcast(m1[:], T - 2),
                                op=mybir.AluOpType.is_ge)
        w2 = small_pool.tile([1, TS], _BF16)
        w2v = w2[:].rearrange("p (s t) -> p s t", s=S)[:, :, 0:T - 2]
        nc.vector.tensor_tensor(w2v, ssv27, _bcast(m2[:], T - 2),
                                op=mybir.AluOpType.is_ge)
        wQ = small_pool.tile([1, TS], _BF16)
        wQv = wQ[:].rearrange("p (s t) -> p s t", s=S)[:, :, 0:T - 2]
        nc.vector.tensor_tensor(wQv, w2v, w1v, op=mybir.AluOpType.subtract)

        wb1_psum = psum_pool.tile([128, CB * TS], _F32, tag="wb1")
        wbQ_psum = psum_pool.tile([128, CB * TS], _F32, tag="wbQ")
        for cb in range(CB):
            wb1s = sview(wb1_psum, cb, 0, T - 2)
            wbQs = sview(wbQ_psum, cb, 0, T - 2)
            nc.tensor.matmul(wb1s, lhsT=wrow[:], rhs=w1v, start=True, stop=True)
            nc.tensor.matmul(wbQs, lhsT=wrow[:], rhs=wQv, start=True, stop=True)

        def wpass(wb, dst):
            prod = small_pool.tile([128, CB * TS], _F32, tag="prod")
            pv = prod[:].rearrange("p (b s t) -> p b s t", b=CB, s=S)[:, :, :, 0:T - 2]
            fv = feats[:].rearrange("p (b s t) -> p b s t", b=CB, s=S)[:, :, :, 0:T - 2]
            wv = wb[:].rearrange("p (b s t) -> p b s t", b=CB, s=S)[:, :, :, 0:T - 2]
            nc.vector.scalar_tensor_tensor(
                pv, fv, WSCALE, wv,
                op0=mybir.AluOpType.mult, op1=mybir.AluOpType.mult,
            )
            nc.vector.tensor_reduce(
                dst[:].rearrange("p (b s) -> p b s", b=CB), pv,
                axis=mybir.AxisListType.X, op=mybir.AluOpType.add,
            )

        P = small_pool.tile([128, CB * S], _F32)
        Q = small_pool.tile([128, CB * S], _F32)
        wpass(wb1_psum, P)
        wpass(wbQ_psum, Q)

        # ---- frame 28 streams now; merge it into the candidates ----
        for cb in range(CB):
            tl = in_pool.tile([128, 4 * S * GROUP], _F32, name="tl", tag="tl")
            nc.sync.dma_start(
                tl[:, :S * GROUP],
                x[cb * 128:(cb + 1) * 128, (T - 2) * FRAME:(T - 1) * FRAME],
            )
            stripe_sums(cb, T - 2, tl, 0, S, SPLIT_ST[(cb, T - 2)])
            nc.vector.tensor_tensor(
                sqcols(T - 2, cb), fcols(T - 2, cb), fcols(T - 2, cb),
                op=mybir.AluOpType.mult,
            )
        for cb in range(CB):
            nc.tensor.matmul(
                ssv[:, :, T - 2:T - 1], lhsT=ones_col[:],
                rhs=sqcols(T - 2, cb).rearrange("p (x o) -> p x o", o=1),
                start=(cb == 0), stop=(cb == 1),
            )

        PQ = small_pool.tile([128, CB * S], _F32)
        nc.vector.tensor_tensor(PQ[:], P[:], Q[:], op=mybir.AluOpType.add)

        # (masks are exact 0.0/1.0 floats, so arithmetic blending is exact)
        s28 = ss_psum[:, T - 2:TS:T]  # (1, 8) stepped view
        h28 = small_pool.tile([128, CB * S], _F32)
        nc.vector.tensor_scalar_mul(h28[:], fcols(T - 2), WSCALE)
        hmQ = small_pool.tile([128, CB * S], _F32)
        nc.vector.tensor_tensor(hmQ[:], h28[:], Q[:], op=mybir.AluOpType.subtract)
        c28a = small_pool.tile([1, S], _F32)
        nc.vector.tensor_tensor(c28a[:], s28, m2[:], op=mybir.AluOpType.is_gt)
        c28b = small_pool.tile([1, S], _F32)
        nc.vector.tensor_tensor(c28b[:], s28, m1[:], op=mybir.AluOpType.is_gt)

        mask28 = psum_pool.tile([128, 2 * CB * S], _F32, tag="mAB")
        for cb in range(CB):
            nc.tensor.matmul(mask28[:, cb * S:(cb + 1) * S],
                             lhsT=ones_row[:], rhs=c28a[:],
                             start=True, stop=True, skip_group_check=True)
            nc.tensor.matmul(mask28[:, CB * S + cb * S:CB * S + (cb + 1) * S],
                             lhsT=ones_row[:], rhs=c28b[:],
                             start=True, stop=True, skip_group_check=True)

        # v0 = P + Q + mask_a*(h28 - Q);  u = P + mask_b*(h28 - P)
        t1 = small_pool.tile([128, CB * S], _F32)
        nc.vector.tensor_tensor(t1[:], mask28[:, 0:CB * S], hmQ[:],
                                op=mybir.AluOpType.mult)
        v0 = small_pool.tile([128, CB * S], _F32)
        nc.vector.tensor_tensor(v0[:], PQ[:], t1[:], op=mybir.AluOpType.add)

        # out_sb prefilled with v0^T: rows (cb, s), cols c
        trv0 = psum_pool.tile([CB * S, 128], _F32, tag="tr")
        nc.tensor.transpose(trv0[:], v0[:], identity[:])
        out_sb_t = small_pool.tile([CB * S, 128], _F32, tag="outsb")
        out_sb = out_sb_t[:]
        nc.scalar.copy(out_sb, trv0[:])

        # off the critical path (Pool where PSUM isn't read): u, uv, merged max
        hmP = small_pool.tile([128, CB * S], _F32)
        nc.gpsimd.tensor_tensor(hmP[:], h28[:], P[:], op=mybir.AluOpType.subtract)
        t2 = small_pool.tile([128, CB * S], _F32)
        nc.vector.tensor_tensor(t2[:], mask28[:, CB * S:], hmP[:],
                                op=mybir.AluOpType.mult)
        u = small_pool.tile([128, CB * S], _F32)
        nc.gpsimd.tensor_tensor(u[:], P[:], t2[:], op=mybir.AluOpType.add)
        uv = small_pool.tile([128, CB * S], _F32)
        nc.gpsimd.tensor_tensor(uv[:], u[:], v0[:], op=mybir.AluOpType.subtract)
        n1 = small_pool.tile([1, S], _F32)
        nc.vector.tensor_tensor(n1[:], m1[:], s28, op=mybir.AluOpType.min)
        m2f = small_pool.tile([1, S], _F32)
        nc.vector.tensor_tensor(m2f[:], m2[:], n1[:], op=mybir.AluOpType.max)

        # ---- frame-29 pieces (DMAs overlap everything above) ----
        # per-stripe sumsq accumulates cb0 (start) + cb1 (stop) in PSUM
        ss29 = psum_pool.tile([1, S], _F32, tag="ss29")
        for pi, (cb, s0, sn, kv) in enumerate(F29_PIECES):
            tl = f29_pool.tile([128, sn * GROUP], _F32, name=f"f29_{pi}")
            nc.sync.dma_start(
                tl[:, :sn * GROUP],
                x[cb * 128:(cb + 1) * 128,
                  (T - 1) * FRAME + s0 * GROUP:(T - 1) * FRAME + (s0 + sn) * GROUP],
            )
            last = pi == len(F29_PIECES) - 1
            if last:
                stripe_sums(cb, T - 1, tl, s0, sn, sn)  # all on DVE, in-tail
                nc.vector.tensor_tensor(
                    sqcols(T - 1, cb, s0, sn), fcols(T - 1, cb, s0, sn),
                    fcols(T - 1, cb, s0, sn), op=mybir.AluOpType.mult,
                )
            else:
                stripe_sums(cb, T - 1, tl, s0, sn, kv)
                nc.gpsimd.tensor_tensor(
                    sqcols(T - 1, cb, s0, sn), fcols(T - 1, cb, s0, sn),
                    fcols(T - 1, cb, s0, sn), op=mybir.AluOpType.mult,
                )
            nc.tensor.matmul(
                ss29[:, s0:s0 + sn], lhsT=ones_col[:],
                rhs=sqcols(T - 1, cb, s0, sn).rearrange("p (x o) -> p x o", o=1),
                start=(cb == 0), stop=(cb == 1), skip_group_check=True,
            )

        # e = (u - v0) + 0.5*mean(f29): the delta applied where frame 29 wins
        e = small_pool.tile([128, CB * S], _F32)
        nc.vector.scalar_tensor_tensor(
            e[:], fcols(T - 1), WSCALE, uv[:],
            op0=mybir.AluOpType.mult, op1=mybir.AluOpType.add,
        )
        tail_ps = psum_pool.tile([CB * S, 132], _F32, tag="tail")
        tre = tail_ps[:, 0:128]
        maskT = tail_ps[:, 128:129]
        nc.tensor.transpose(tre, e[:], identity[:])

        # per-stripe decision: does frame 29 beat the 0..28 second max?
        # (computed directly as a (1, 16) cb-replicated row)
        mask16 = small_pool.tile([1, CB * S], _F32)
        nc.vector.tensor_tensor(
            mask16[:].rearrange("p (b s) -> p b s", b=CB),
            _cbrep(ss29[:]), _cbrep(m2f[:]), op=mybir.AluOpType.is_gt)
        nc.tensor.matmul(maskT, lhsT=mask16[:], rhs=ones_col[0:1, :],
                         start=True, stop=True, skip_group_check=True)
        maskTs = small_pool.tile([CB * S, 1], _F32)
        nc.vector.tensor_copy(maskTs[:], maskT)

        # blend: out = v0^T + mask * e^T (single fused op), then store
        nc.vector.scalar_tensor_tensor(
            out_sb, tre, maskTs[:], out_sb,
            op0=mybir.AluOpType.mult, op1=mybir.AluOpType.add,
        )
        nc.sync.dma_start(out.rearrange("s (b c) -> b s c", b=CB), out_sb)


_NC_CACHE = {}


def _get_nc(repeat=1):
    if repeat not in _NC_CACHE:
        nc = bacc.Bacc("TRN2", target_bir_lowering=False, debug=False)
        x = nc.dram_tensor("x", [C, T * FRAME], _F32, kind="ExternalInput").ap()
        out = nc.dram_tensor("out", [S, C], _F32, kind="ExternalOutput").ap()
        with tile.TileContext(nc) as tc:
            with ExitStack() as ctx:
                _kernel_body(ctx, tc, nc, x, out, repeat=repeat)
        nc.compile()
        _NC_CACHE[repeat] = nc
    return _NC_CACHE[repeat]


def kernel(x):
    x = np.asarray(x, dtype=np.float32)
    assert x.shape == (N, C, T, H, W), x.shape
    nc = _get_nc()
    in_maps = [{"x": np.ascontiguousarray(x[i]).reshape(C, T * FRAME)} for i in range(N)]
    res = run_bass_kernel_spmd(nc, in_maps, list(range(N)))
    return np.stack([res.results[i]["out"].reshape(S * C) for i in range(N)])
